# revision 3
# baseline (speedup 1.0000x reference)
import sys
sys.path.insert(0, '/opt/trn_rl_repo')

"""Multi-head attention TP kernel for TRN2 — per-core program builder.

Sharding: 8 cores = 2 (batch) x 4 (head groups of 4 heads = 512 dims).
Each core computes, for its batch b and head-dim slice e:
    q = x[b] @ wq[e,:].T + bq[e]      (stored transposed: qT [E, S])
    k = x[b] @ wk[e,:].T + bk[e]      (kT [E, S])
    v = x[b] @ wv[e,:].T + bv[e]      (v [S, E])
    per head h (dh=128): ST = K-major score tiles, exp (no max-sub; scores
    bounded ~|3|), softmax denominator via all-ones broadcast matmul,
    AV accumulated unnormalized, normalized on eviction.
    partial_out = attn_out @ wo[:, e].T   ([S, D]; host sums 8 partials + bo)

Data path is bf16 (PE full rate); accumulation fp32 in PSUM. x is shipped
bf16 s-chunk-major and processed through a rolling 3-deep SBUF buffer:
Q/K/V projections for s-chunk si all run while chunk si+1 streams in, so
the first matmul only waits for wq+wk+chunk0 (~6.3MB). PSUM evictions run
on DVE (tensor_scalar_add fuses the bias); exp runs on the scalar engine
in N=1024 batches (2 PSUM banks per ACTIVATE) to amortize instruction
overhead; softmax normalization uses reciprocal_approx_fast. Output is
written bf16 (host accumulates partials in fp32).
"""

import math

import numpy as np

import concourse.bass as bass
import concourse.tile as tile
from concourse import bacc, mybir

F32 = mybir.dt.float32
BF16 = mybir.dt.bfloat16
AF = mybir.ActivationFunctionType


def build_module(
    S=2048,          # sequence per core (one batch)
    D=2048,          # model dim
    E=512,           # head dims per core (4 heads x 128)
    bufs_es=3,
    enable_asserts=False,
):
    HD = 128
    SC = 512
    NK = D // HD        # proj contraction tiles
    NH = E // HD        # heads per core
    NSC = S // SC       # s-chunks / i-blocks
    NJ = S // HD        # attention j tiles
    NJG = NJ // 2       # j-tile pairs (exp batches)
    ND = D // SC        # WO n-chunks
    NIT = S // HD       # WO i tiles
    scale = 1.0 / math.sqrt(HD)

    nc = bacc.Bacc(
        "TRN2",
        target_bir_lowering=False,
        debug=False,
        enable_asserts=enable_asserts,
        num_devices=8,
    )

    # x is s-chunk-major: [HD, NSC * NK * SC]
    xr = nc.dram_tensor("xr", [HD, NSC * NK * SC], BF16,
                        kind="ExternalInput").ap()
    wqt = nc.dram_tensor("wqt", [HD, NK * E], BF16, kind="ExternalInput").ap()
    wkt = nc.dram_tensor("wkt", [HD, NK * E], BF16, kind="ExternalInput").ap()
    wvt = nc.dram_tensor("wvt", [HD, NK * E], BF16, kind="ExternalInput").ap()
    wot = nc.dram_tensor("wot", [HD, NH * D], BF16, kind="ExternalInput").ap()
    bqc = nc.dram_tensor("bqc", [HD, NH], F32, kind="ExternalInput").ap()
    bkc = nc.dram_tensor("bkc", [HD, NH], F32, kind="ExternalInput").ap()
    bvr = nc.dram_tensor("bvr", [1, E], BF16, kind="ExternalInput").ap()
    ones2d = nc.dram_tensor("ones2d", [HD, HD], BF16,
                            kind="ExternalInput").ap()
    out = nc.dram_tensor("out", [S, D], BF16, kind="ExternalOutput").ap()

    with tile.TileContext(nc) as tc:
        with (
            tc.tile_pool(name="qkv", bufs=1) as qkv_pool,
            tc.tile_pool(name="consts", bufs=1) as consts,
            tc.tile_pool(name="wbuf", bufs=1) as w_pool,
            tc.tile_pool(name="xroll", bufs=3) as x_pool,
        ):
            q_sb = qkv_pool.tile([HD, NH, S], BF16)
            k_sb = qkv_pool.tile([HD, NH, S], BF16)
            v_sb = qkv_pool.tile([HD, NJ, E], BF16)

            wq_sb = w_pool.tile([HD, NK, E], BF16, tag="wq")
            wk_sb = w_pool.tile([HD, NK, E], BF16, tag="wk")
            wv_sb = w_pool.tile([HD, NK, E], BF16, tag="wv")
            wo_sb = w_pool.tile([HD, NH, D], BF16, tag="wo")

            bq_sb = consts.tile([HD, NH], F32)
            bk_sb = consts.tile([HD, NH], F32)
            bv_sb = consts.tile([1, E], BF16)
            allones = consts.tile([HD, HD], BF16)    # bcast-sum stationary

            # ---- DMA issue order = need order: wq, wk, x0, consts, wv,
            # x1..x3, wo.  All on one HWDGE trigger queue -> FIFO drain in
            # this order, so the first QK matmul waits only ~6.3MB.
            def load_w(dst, src, width):
                for g4 in range(NK // 4):
                    nc.sync.dma_start(
                        out=dst[:, 4 * g4:4 * g4 + 4, :],
                        in_=src[:, 4 * g4 * width:(4 * g4 + 4) * width]
                        .rearrange("p (k e) -> p k e", e=width))

            x_tiles = []

            def load_x(si):
                xt = x_pool.tile([HD, NK, SC], BF16, tag="x", name=f"x{si}")
                for g in range(2):
                    off = si * NK * SC + g * (NK // 2) * SC
                    nc.sync.dma_start(
                        out=xt[:, g * (NK // 2):(g + 1) * (NK // 2), :],
                        in_=xr[:, off:off + (NK // 2) * SC].rearrange(
                            "p (k s) -> p k s", s=SC))
                return xt

            load_w(wq_sb, wqt, E)
            load_w(wk_sb, wkt, E)
            x_tiles.append(load_x(0))
            nc.sync.dma_start(out=bq_sb, in_=bqc)
            nc.sync.dma_start(out=bk_sb, in_=bkc)
            nc.sync.dma_start(out=bv_sb, in_=bvr)
            nc.sync.dma_start(out=allones, in_=ones2d)
            load_w(wv_sb, wvt, E)
            for si in range(1, NSC):
                x_tiles.append(load_x(si))
            nc.sync.dma_start(
                out=wo_sb, in_=wot.rearrange("p (k d) -> p k d", d=D))

            # -------- Phase A: fused Q,K,V projections per s-chunk --------
            with tc.tile_pool(name="psAB", bufs=1, space="PSUM") as psAB:
                for si in range(NSC):
                    s0 = si * SC
                    xt = x_tiles[si]
                    xv = xt.rearrange("p k (t h) -> p k t h", h=HD)
                    psQ = [psAB.tile([HD, SC], F32, tag=f"q{m}",
                                     name=f"psq{m}") for m in range(NH)]
                    psK = [psAB.tile([HD, SC], F32, tag=f"k{m}",
                                     name=f"psk{m}") for m in range(NH)]
                    for kk in range(NK):
                        st = kk == 0
                        sp = kk == NK - 1
                        for m in range(NH):
                            nc.tensor.matmul(
                                psQ[m],
                                wq_sb[:, kk, m * HD:(m + 1) * HD],
                                xt[:, kk, :],
                                start=st, stop=sp,
                            )
                            nc.tensor.matmul(
                                psK[m],
                                wk_sb[:, kk, m * HD:(m + 1) * HD],
                                xt[:, kk, :],
                                start=st, stop=sp,
                            )
                    for m in range(NH):
                        nc.vector.tensor_scalar_add(
                            q_sb[:, m, s0:s0 + SC], psQ[m],
                            bq_sb[:, m:m + 1])
                        nc.vector.tensor_scalar_add(
                            k_sb[:, m, s0:s0 + SC], psK[m],
                            bk_sb[:, m:m + 1])
                    # V for this chunk: x stationary, wv moving
                    psV = [psAB.tile([HD, E], F32, tag=f"q{mv}",
                                     name=f"psv{mv}") for mv in range(NH)]
                    for kk in range(NK):
                        for mv in range(NH):
                            nc.tensor.matmul(
                                psV[mv],
                                xv[:, kk, mv, :],
                                wv_sb[:, kk, :],
                                start=(kk == 0), stop=False,
                            )
                    for mv in range(NH):
                        # bias: ones^T @ bv_row as a final K=1 matmul
                        nc.tensor.matmul(
                            psV[mv], allones[0:1, :], bv_sb,
                            start=False, stop=True,
                        )
                        nc.vector.tensor_copy(v_sb[:, si * NH + mv, :],
                                              psV[mv])

            # ---------------- Phase C: attention ----------------
            with tc.tile_pool(name="outT", bufs=1) as outT_pool:
                outT_sb = outT_pool.tile([HD, NH, S], BF16)
                with (
                    tc.tile_pool(name="attws", bufs=2) as ws_pool,
                    tc.tile_pool(name="es", bufs=bufs_es) as es_pool,
                    tc.tile_pool(name="psS", bufs=2, space="PSUM") as psS_pool,
                    tc.tile_pool(name="psO", bufs=2, space="PSUM") as psO_pool,
                    tc.tile_pool(name="psN", bufs=2, space="PSUM") as psN_pool,
                ):
                    for h in range(NH):
                        for ib in range(NSC):
                            i0 = ib * SC
                            psO = psO_pool.tile([HD, SC], F32, tag="o")
                            ps_bc = psN_pool.tile([HD, SC], F32, tag="bc")
                            es_t = [None] * NJG

                            def emit_av(jg, h=h, psO=psO, ps_bc=ps_bc,
                                        es_t=es_t):
                                for u in range(2):
                                    j = 2 * jg + u
                                    nc.tensor.matmul(
                                        psO,
                                        v_sb[:, j, h * HD:(h + 1) * HD],
                                        es_t[jg][:, u, :],
                                        start=(j == 0), stop=(j == NJ - 1),
                                    )
                                    nc.tensor.matmul(
                                        ps_bc, allones, es_t[jg][:, u, :],
                                        start=(j == 0), stop=(j == NJ - 1),
                                    )

                            for jg in range(NJG):
                                psS = psS_pool.tile([HD, 2, SC], F32, tag="s")
                                for u in range(2):
                                    j = 2 * jg + u
                                    nc.tensor.matmul(
                                        psS[:, u, :],
                                        k_sb[:, h, j * HD:(j + 1) * HD],
                                        q_sb[:, h, i0:i0 + SC],
                                        start=True, stop=True,
                                    )
                                if jg >= 1:
                                    emit_av(jg - 1)
                                es = es_pool.tile([HD, 2, SC], BF16, tag="es",
                                                  name="es")
                                nc.scalar.activation(es, psS, AF.Exp,
                                                     scale=scale)
                                es_t[jg] = es
                            emit_av(NJG - 1)
                            recip = ws_pool.tile([HD, SC], F32, tag="recip")
                            nc.vector.reciprocal_approx_fast(recip, ps_bc)
                            nc.vector.tensor_mul(
                                outT_sb[:, h, i0:i0 + SC], psO, recip)

                # ---------------- Phase D: WO projection ----------------
                with (
                    tc.tile_pool(name="og", bufs=2) as og_pool,
                    tc.tile_pool(name="psW", bufs=4, space="PSUM") as psW_pool,
                ):
                    for it in range(NIT):
                        og = og_pool.tile([HD, D], BF16, tag="og")
                        for nn in range(ND):
                            psW = psW_pool.tile([HD, SC], F32, tag="w")
                            for kk in range(NH):
                                nc.tensor.matmul(
                                    psW,
                                    outT_sb[:, kk, it * HD:(it + 1) * HD],
                                    wo_sb[:, kk, nn * SC:(nn + 1) * SC],
                                    start=(kk == 0), stop=(kk == NH - 1),
                                )
                            if nn % 2 == 0:
                                nc.scalar.copy(
                                    og[:, nn * SC:(nn + 1) * SC], psW)
                            else:
                                nc.vector.tensor_copy(
                                    og[:, nn * SC:(nn + 1) * SC], psW)
                        nc.sync.dma_start(
                            out=out[it * HD:(it + 1) * HD, :], in_=og)

    nc.compile()
    return nc


# ---------------------------------------------------------------------------
# Host-side sharding helpers
# ---------------------------------------------------------------------------

def _bf16(a):
    import ml_dtypes
    return np.asarray(a).astype(ml_dtypes.bfloat16)


def make_in_map(x_b, wq_e, bq_e, wk_e, bk_e, wv_e, bv_e, wo_e):
    """Per-core input dict. x_b [S, D]; w*_e [E, D] row slices; wo_e [D, E]
    column slice; b*_e [E]."""
    E = wq_e.shape[0]
    S, D = x_b.shape
    HD = 128
    SC = 512
    NH = E // HD
    NK = D // HD
    NSC = S // SC

    def wrelayout(wT):  # [D, E'] -> [HD, NK*E'] with k-tile-major columns
        Ew = wT.shape[1]
        return _bf16(
            wT.reshape(NK, HD, Ew).transpose(1, 0, 2).reshape(HD, NK * Ew))

    xT = x_b.T  # [D, S]
    # s-chunk-major x: xr[hd, si, k, s] = xT[k*HD+hd, si*SC+s]
    xr = xT.reshape(NK, HD, NSC, SC).transpose(1, 2, 0, 3).reshape(HD, -1)
    return {
        "xr": _bf16(xr),
        "wqt": wrelayout(wq_e.T),
        "wkt": wrelayout(wk_e.T),
        "wvt": wrelayout(wv_e.T),
        "wot": _bf16(
            wo_e.T.reshape(NH, HD, D).transpose(1, 0, 2).reshape(HD, NH * D)),
        "bqc": np.ascontiguousarray(bq_e.reshape(NH, HD).T),
        "bkc": np.ascontiguousarray(bk_e.reshape(NH, HD).T),
        "bvr": _bf16(bv_e.reshape(1, E)),
        "ones2d": _bf16(np.ones((HD, HD), np.float32)),
    }


def core_reference(x_b, wq_e, bq_e, wk_e, bk_e, wv_e, bv_e, wo_e):
    """Numpy reference for one core's partial output."""
    HD = 128
    q = x_b @ wq_e.T + bq_e
    k = x_b @ wk_e.T + bk_e
    v = x_b @ wv_e.T + bv_e
    E = q.shape[1]
    outs = []
    for h in range(E // HD):
        qh = q[:, h * HD:(h + 1) * HD]
        kh = k[:, h * HD:(h + 1) * HD]
        vh = v[:, h * HD:(h + 1) * HD]
        s = (qh @ kh.T) / math.sqrt(HD)
        p = np.exp(s)
        outs.append((p @ vh) / p.sum(-1, keepdims=True))
    o = np.concatenate(outs, axis=1)
    return o @ wo_e.T


# ---------------------------------------------------------------------------
# Entry point: full-input kernel with internal 8-way sharding
# ---------------------------------------------------------------------------

import os as _os

_NC_CACHE = {}


def _get_module():
    if "nc" not in _NC_CACHE:
        _NC_CACHE["nc"] = build_module(S=2048, D=2048, E=512)
    return _NC_CACHE["nc"]


def kernel(x, wq, bq, wk, bk, wv, bv, wo, bo):
    """Full inputs -> full output. 8 cores = 2 (batch) x 4 (head-group)."""
    from concourse import bass_utils

    x = np.asarray(x, dtype=np.float32)
    wq, bq = np.asarray(wq, np.float32), np.asarray(bq, np.float32)
    wk, bk = np.asarray(wk, np.float32), np.asarray(bk, np.float32)
    wv, bv = np.asarray(wv, np.float32), np.asarray(bv, np.float32)
    wo, bo = np.asarray(wo, np.float32), np.asarray(bo, np.float32)

    E = 512
    nc = _get_module()
    in_maps = []
    for c in range(8):
        b, g = divmod(c, 4)
        e = slice(g * E, (g + 1) * E)
        in_maps.append(make_in_map(
            x[b], wq[e], bq[e], wk[e], bk[e], wv[e], bv[e], wo[:, e]))

    trace = bool(int(_os.environ.get("ATTN_TRACE", "0")))
    kw = {}
    if trace:
        tmpdir = _os.environ.get("ATTN_TRACE_DIR") or None
        kw = dict(trace=True, tmpdir=tmpdir, trace_cores=[0])
    res = bass_utils.run_bass_kernel_spmd(
        nc, in_maps, core_ids=list(range(8)), **kw)
    if trace:
        print(f"HW exec time: {res.exec_time_ns} ns")
        _NC_CACHE["last_results"] = res

    y = np.empty((2, 2048, 2048), np.float32)
    for b in range(2):
        acc = np.asarray(res.results[4 * b]["out"], np.float32)
        for g in range(1, 4):
            acc += np.asarray(res.results[4 * b + g]["out"], np.float32)
        y[b] = acc + bo
    return y


# revision 4
# speedup vs baseline: 1.0787x; 1.0787x over previous
import sys
sys.path.insert(0, '/opt/trn_rl_repo')

"""Multi-head attention TP kernel for TRN2 — per-core program builder.

Sharding: 8 cores = 2 (batch) x 4 (head groups of 4 heads = 512 dims).
Each core computes, for its batch b and head-dim slice e:
    q = x[b] @ wq[e,:].T + bq[e]      (stored transposed: qT [E, S])
    k = x[b] @ wk[e,:].T + bk[e]      (kT [E, S])
    v = x[b] @ wv[e,:].T + bv[e]      (v [S, E])
    per head h (dh=128): ST = K-major score tiles, exp (no max-sub; scores
    bounded ~|3|), AV accumulated unnormalized.  Softmax denominator via
    DVE accumulation of the exp tiles + gpsimd partition all-reduce (no PE
    cycles).  Normalized on eviction with reciprocal_approx_fast.
    partial_out = attn_out @ wo[:, e].T   ([S, D]; host sums 8 partials + bo)

The kernel is PE-cycle-bound (the PE power-throttles to ~2.0 GHz when
saturated), so everything that is not a projection/score/AV matmul is kept
off the tensor engine: biases via DVE tensor_scalar / broadcast adds, exp
on the scalar engine in N=1024 batches, denominator on DVE+gpsimd.  WO is
interleaved per i-block so its matmuls fill attention dependency gaps.
x streams through a rolling 2-chunk buffer; DMAs are issued in k-group
interleaved order so the first matmul starts after ~1.6MB has landed.
Output is bf16 (host accumulates partials in fp32).
"""

import math

import numpy as np

import concourse.bass as bass
import concourse.tile as tile
from concourse import bacc, mybir

F32 = mybir.dt.float32
BF16 = mybir.dt.bfloat16
AF = mybir.ActivationFunctionType


def build_module(
    S=2048,          # sequence per core (one batch)
    D=2048,          # model dim
    E=512,           # head dims per core (4 heads x 128)
    bufs_es=4,
    enable_asserts=False,
):
    HD = 128
    SC = 512
    NK = D // HD        # proj contraction tiles
    NH = E // HD        # heads per core
    NSC = S // SC       # s-chunks / i-blocks
    NJ = S // HD        # attention j tiles
    NJG = NJ // 2       # j-tile pairs (exp batches)
    ND = D // SC        # WO n-chunks
    scale = 1.0 / math.sqrt(HD)

    nc = bacc.Bacc(
        "TRN2",
        target_bir_lowering=False,
        debug=False,
        enable_asserts=enable_asserts,
        num_devices=8,
    )

    # x is s-chunk-major: [HD, NSC * NK * SC]
    xr = nc.dram_tensor("xr", [HD, NSC * NK * SC], BF16,
                        kind="ExternalInput").ap()
    wqt = nc.dram_tensor("wqt", [HD, NK * E], BF16, kind="ExternalInput").ap()
    wkt = nc.dram_tensor("wkt", [HD, NK * E], BF16, kind="ExternalInput").ap()
    wvt = nc.dram_tensor("wvt", [HD, NK * E], BF16, kind="ExternalInput").ap()
    wot = nc.dram_tensor("wot", [HD, NH * D], BF16, kind="ExternalInput").ap()
    bqc = nc.dram_tensor("bqc", [HD, NH], F32, kind="ExternalInput").ap()
    bkc = nc.dram_tensor("bkc", [HD, NH], F32, kind="ExternalInput").ap()
    bvr = nc.dram_tensor("bvr", [1, E], BF16, kind="ExternalInput").ap()
    out = nc.dram_tensor("out", [S, D], BF16, kind="ExternalOutput").ap()

    with tile.TileContext(nc) as tc:
        with (
            tc.tile_pool(name="qkv", bufs=1) as qkv_pool,
            tc.tile_pool(name="consts", bufs=1) as consts,
            tc.tile_pool(name="wbuf", bufs=1) as w_pool,
            tc.tile_pool(name="xroll", bufs=2) as x_pool,
        ):
            q_sb = qkv_pool.tile([HD, NH, S], BF16)
            k_sb = qkv_pool.tile([HD, NH, S], BF16)
            v_sb = qkv_pool.tile([HD, NJ, E], BF16)

            wq_sb = w_pool.tile([HD, NK, E], BF16, tag="wq")
            wk_sb = w_pool.tile([HD, NK, E], BF16, tag="wk")
            wv_sb = w_pool.tile([HD, NK, E], BF16, tag="wv")
            wo_sb = w_pool.tile([HD, NH, D], BF16, tag="wo")

            bq_sb = consts.tile([HD, NH], F32)
            bk_sb = consts.tile([HD, NH], F32)
            bv_sb = consts.tile([1, E], BF16)
            bv_bc = consts.tile([HD, E], BF16)

            # ---- DMA issue order = need order.  One HWDGE trigger queue
            # drains FIFO, so interleave per k-group: the kk=0..3 matmuls
            # of s-chunk 0 only need the first ~1.6MB.
            NG = NK // 4

            def load_w_g(dst, src, width, g):
                nc.sync.dma_start(
                    out=dst[:, 4 * g:4 * g + 4, :],
                    in_=src[:, 4 * g * width:(4 * g + 4) * width]
                    .rearrange("p (k e) -> p k e", e=width))

            x_tiles = [None] * NSC

            def load_x_g(si, g):
                if x_tiles[si] is None:
                    x_tiles[si] = x_pool.tile([HD, NK, SC], BF16, tag="x",
                                              name=f"x{si}")
                off = si * NK * SC + 4 * g * SC
                nc.sync.dma_start(
                    out=x_tiles[si][:, 4 * g:4 * g + 4, :],
                    in_=xr[:, off:off + 4 * SC].rearrange(
                        "p (k s) -> p k s", s=SC))

            for g in range(NG):
                load_w_g(wq_sb, wqt, E, g)
                load_w_g(wk_sb, wkt, E, g)
                load_x_g(0, g)
            nc.sync.dma_start(out=bq_sb, in_=bqc)
            nc.sync.dma_start(out=bk_sb, in_=bkc)
            nc.sync.dma_start(out=bv_sb, in_=bvr)
            for g in range(NG):
                load_w_g(wv_sb, wvt, E, g)
            for g in range(NG):
                load_x_g(1, g)
            nc.sync.dma_start(
                out=wo_sb, in_=wot.rearrange("p (k d) -> p k d", d=D))
            nc.gpsimd.partition_broadcast(bv_bc, bv_sb)

            # -------- Phase A: fused Q,K,V projections per s-chunk --------
            with tc.tile_pool(name="psAB", bufs=1, space="PSUM") as psAB:
                for si in range(NSC):
                    s0 = si * SC
                    xt = x_tiles[si]
                    xv = xt.rearrange("p k (t h) -> p k t h", h=HD)
                    psQ = [psAB.tile([HD, SC], F32, tag=f"q{m}",
                                     name=f"psq{m}") for m in range(NH)]
                    psK = [psAB.tile([HD, SC], F32, tag=f"k{m}",
                                     name=f"psk{m}") for m in range(NH)]
                    for kk in range(NK):
                        st = kk == 0
                        sp = kk == NK - 1
                        for m in range(NH):
                            nc.tensor.matmul(
                                psQ[m],
                                wq_sb[:, kk, m * HD:(m + 1) * HD],
                                xt[:, kk, :],
                                start=st, stop=sp,
                            )
                            nc.tensor.matmul(
                                psK[m],
                                wk_sb[:, kk, m * HD:(m + 1) * HD],
                                xt[:, kk, :],
                                start=st, stop=sp,
                            )
                    for m in range(NH):
                        nc.vector.tensor_scalar_add(
                            q_sb[:, m, s0:s0 + SC], psQ[m],
                            bq_sb[:, m:m + 1])
                        nc.vector.tensor_scalar_add(
                            k_sb[:, m, s0:s0 + SC], psK[m],
                            bk_sb[:, m:m + 1])
                    # prefetch x for si+2 now that chunk si is nearly done
                    if si + 2 < NSC:
                        for g in range(NG):
                            load_x_g(si + 2, g)
                    # V for this chunk: x stationary, wv moving
                    psV = [psAB.tile([HD, E], F32, tag=f"q{mv}",
                                     name=f"psv{mv}") for mv in range(NH)]
                    for kk in range(NK):
                        for mv in range(NH):
                            nc.tensor.matmul(
                                psV[mv],
                                xv[:, kk, mv, :],
                                wv_sb[:, kk, :],
                                start=(kk == 0), stop=(kk == NK - 1),
                            )
                    for mv in range(NH):
                        nc.vector.tensor_add(
                            v_sb[:, si * NH + mv, :], psV[mv], bv_bc)

            # ------- Phase C/D: attention with WO interleaved per ib -------
            with (
                tc.tile_pool(name="outT", bufs=1) as outT_pool,
                tc.tile_pool(name="attws", bufs=2) as ws_pool,
                tc.tile_pool(name="es", bufs=bufs_es) as es_pool,
                tc.tile_pool(name="og", bufs=2) as og_pool,
                tc.tile_pool(name="psS", bufs=2, space="PSUM") as psS_pool,
                tc.tile_pool(name="psO", bufs=2, space="PSUM") as psO_pool,
                tc.tile_pool(name="psW", bufs=2, space="PSUM") as psW_pool,
            ):
                outT_sb = outT_pool.tile([HD, NH, S], BF16)
                for ib in range(NSC):
                    i0 = ib * SC
                    for h in range(NH):
                        psO = psO_pool.tile([HD, SC], F32, tag="o")
                        acc_a = ws_pool.tile([HD, SC], F32, tag="acca")
                        acc_b = ws_pool.tile([HD, SC], F32, tag="accb")
                        es_t = [None] * NJG

                        def emit_av(jg, h=h, psO=psO, es_t=es_t):
                            for u in range(2):
                                j = 2 * jg + u
                                nc.tensor.matmul(
                                    psO,
                                    v_sb[:, j, h * HD:(h + 1) * HD],
                                    es_t[jg][:, u, :],
                                    start=(j == 0), stop=(j == NJ - 1),
                                )

                        for jg in range(NJG):
                            psS = psS_pool.tile([HD, 2, SC], F32, tag="s")
                            for u in range(2):
                                j = 2 * jg + u
                                nc.tensor.matmul(
                                    psS[:, u, :],
                                    k_sb[:, h, j * HD:(j + 1) * HD],
                                    q_sb[:, h, i0:i0 + SC],
                                    start=True, stop=True,
                                )
                            if jg >= 1:
                                emit_av(jg - 1)
                            es = es_pool.tile([HD, 2, SC], BF16, tag="es",
                                              name="es")
                            nc.scalar.activation(es, psS, AF.Exp,
                                                 scale=scale)
                            es_t[jg] = es
                            # softmax-denominator accumulation on DVE,
                            # two independent chains to halve latency
                            acc = acc_a if jg < NJG // 2 else acc_b
                            if jg % (NJG // 2) == 0:
                                nc.vector.tensor_add(
                                    acc, es[:, 0, :], es[:, 1, :])
                            else:
                                nc.vector.tensor_add(acc, acc, es[:, 0, :])
                                nc.vector.tensor_add(acc, acc, es[:, 1, :])
                        emit_av(NJG - 1)
                        nc.vector.tensor_add(acc_a, acc_a, acc_b)
                        denom = ws_pool.tile([HD, SC], F32, tag="den")
                        nc.gpsimd.partition_all_reduce(
                            denom, acc_a, HD, bass.bass_isa.ReduceOp.add)
                        recip = ws_pool.tile([HD, SC], F32, tag="recip")
                        nc.vector.reciprocal_approx_fast(recip, denom)
                        nc.vector.tensor_mul(
                            outT_sb[:, h, i0:i0 + SC], psO, recip)

                    # WO projection for this ib's four i-tiles
                    for t in range(NSC):
                        it = ib * NSC + t
                        og = og_pool.tile([HD, D], BF16, tag="og")
                        for nn in range(ND):
                            psW = psW_pool.tile([HD, SC], F32, tag="w")
                            for kk in range(NH):
                                nc.tensor.matmul(
                                    psW,
                                    outT_sb[:, kk, it * HD:(it + 1) * HD],
                                    wo_sb[:, kk, nn * SC:(nn + 1) * SC],
                                    start=(kk == 0), stop=(kk == NH - 1),
                                )
                            if nn % 2 == 0:
                                nc.scalar.copy(
                                    og[:, nn * SC:(nn + 1) * SC], psW)
                            else:
                                nc.vector.tensor_copy(
                                    og[:, nn * SC:(nn + 1) * SC], psW)
                        nc.sync.dma_start(
                            out=out[it * HD:(it + 1) * HD, :], in_=og)

    nc.compile()
    return nc


# ---------------------------------------------------------------------------
# Host-side sharding helpers
# ---------------------------------------------------------------------------

def _bf16(a):
    import ml_dtypes
    return np.asarray(a).astype(ml_dtypes.bfloat16)


def make_in_map(x_b, wq_e, bq_e, wk_e, bk_e, wv_e, bv_e, wo_e):
    """Per-core input dict. x_b [S, D]; w*_e [E, D] row slices; wo_e [D, E]
    column slice; b*_e [E]."""
    E = wq_e.shape[0]
    S, D = x_b.shape
    HD = 128
    SC = 512
    NH = E // HD
    NK = D // HD
    NSC = S // SC

    def wrelayout(wT):  # [D, E'] -> [HD, NK*E'] with k-tile-major columns
        Ew = wT.shape[1]
        return _bf16(
            wT.reshape(NK, HD, Ew).transpose(1, 0, 2).reshape(HD, NK * Ew))

    xT = x_b.T  # [D, S]
    # s-chunk-major x: xr[hd, si, k, s] = xT[k*HD+hd, si*SC+s]
    xr = xT.reshape(NK, HD, NSC, SC).transpose(1, 2, 0, 3).reshape(HD, -1)
    return {
        "xr": _bf16(xr),
        "wqt": wrelayout(wq_e.T),
        "wkt": wrelayout(wk_e.T),
        "wvt": wrelayout(wv_e.T),
        "wot": _bf16(
            wo_e.T.reshape(NH, HD, D).transpose(1, 0, 2).reshape(HD, NH * D)),
        "bqc": np.ascontiguousarray(bq_e.reshape(NH, HD).T),
        "bkc": np.ascontiguousarray(bk_e.reshape(NH, HD).T),
        "bvr": _bf16(bv_e.reshape(1, E)),
    }


def core_reference(x_b, wq_e, bq_e, wk_e, bk_e, wv_e, bv_e, wo_e):
    """Numpy reference for one core's partial output."""
    HD = 128
    q = x_b @ wq_e.T + bq_e
    k = x_b @ wk_e.T + bk_e
    v = x_b @ wv_e.T + bv_e
    E = q.shape[1]
    outs = []
    for h in range(E // HD):
        qh = q[:, h * HD:(h + 1) * HD]
        kh = k[:, h * HD:(h + 1) * HD]
        vh = v[:, h * HD:(h + 1) * HD]
        s = (qh @ kh.T) / math.sqrt(HD)
        p = np.exp(s)
        outs.append((p @ vh) / p.sum(-1, keepdims=True))
    o = np.concatenate(outs, axis=1)
    return o @ wo_e.T


# ---------------------------------------------------------------------------
# Entry point: full-input kernel with internal 8-way sharding
# ---------------------------------------------------------------------------

import os as _os

_NC_CACHE = {}


def _get_module():
    if "nc" not in _NC_CACHE:
        _NC_CACHE["nc"] = build_module(S=2048, D=2048, E=512)
    return _NC_CACHE["nc"]


def kernel(x, wq, bq, wk, bk, wv, bv, wo, bo):
    """Full inputs -> full output. 8 cores = 2 (batch) x 4 (head-group)."""
    from concourse import bass_utils

    x = np.asarray(x, dtype=np.float32)
    wq, bq = np.asarray(wq, np.float32), np.asarray(bq, np.float32)
    wk, bk = np.asarray(wk, np.float32), np.asarray(bk, np.float32)
    wv, bv = np.asarray(wv, np.float32), np.asarray(bv, np.float32)
    wo, bo = np.asarray(wo, np.float32), np.asarray(bo, np.float32)

    E = 512
    nc = _get_module()
    in_maps = []
    for c in range(8):
        b, g = divmod(c, 4)
        e = slice(g * E, (g + 1) * E)
        in_maps.append(make_in_map(
            x[b], wq[e], bq[e], wk[e], bk[e], wv[e], bv[e], wo[:, e]))

    trace = bool(int(_os.environ.get("ATTN_TRACE", "0")))
    kw = {}
    if trace:
        tmpdir = _os.environ.get("ATTN_TRACE_DIR") or None
        kw = dict(trace=True, tmpdir=tmpdir, trace_cores=[0])
    res = bass_utils.run_bass_kernel_spmd(
        nc, in_maps, core_ids=list(range(8)), **kw)
    if trace:
        print(f"HW exec time: {res.exec_time_ns} ns")
        _NC_CACHE["last_results"] = res

    y = np.empty((2, 2048, 2048), np.float32)
    for b in range(2):
        acc = np.asarray(res.results[4 * b]["out"], np.float32)
        for g in range(1, 4):
            acc += np.asarray(res.results[4 * b + g]["out"], np.float32)
        y[b] = acc + bo
    return y


# revision 9
# speedup vs baseline: 1.2980x; 1.2032x over previous
import sys
sys.path.insert(0, '/opt/trn_rl_repo')

"""Multi-head attention TP kernel for TRN2 — per-core program builder.

Sharding: 8 cores = 2 (batch) x 4 (head groups of 4 heads = 512 dims).
Each core computes, for its batch b and head-dim slice e:
    q = x[b] @ wq[e,:].T + bq[e]      (stored transposed: qT [E, S])
    k = x[b] @ wk[e,:].T + bk[e]      (kT [E, S])
    v = x[b] @ wv[e,:].T + bv[e]      (v [S, E])
    per head h (dh=128): ST = K-major score tiles, exp (no max-sub; scores
    bounded ~|3|), AV accumulated unnormalized.  Softmax denominator via
    DVE accumulation of the exp tiles + gpsimd partition all-reduce (no PE
    cycles).  Normalized on eviction with reciprocal_approx_fast.
    partial_out = attn_out @ wo[:, e].T   ([S, D]; host sums 8 partials + bo)

The kernel is PE-cycle-bound (the PE power-throttles to ~2.0 GHz when
saturated), so everything that is not a projection/score/AV matmul is kept
off the tensor engine: biases via DVE tensor_scalar / broadcast adds, exp
on the scalar engine in N=1024 batches, denominator on DVE+gpsimd.  WO is
interleaved per i-block so its matmuls fill attention dependency gaps.
x streams through a rolling 2-chunk buffer; DMAs are issued in k-group
interleaved order so the first matmul starts after ~1.6MB has landed.
Output is bf16 (host accumulates partials in fp32).
"""

import math

import numpy as np

import concourse.bass as bass
import concourse.tile as tile
from concourse import bacc, mybir

F32 = mybir.dt.float32
BF16 = mybir.dt.bfloat16
AF = mybir.ActivationFunctionType


def build_module(
    S=2048,          # sequence per core (one batch)
    D=2048,          # model dim
    E=512,           # head dims per core (4 heads x 128)
    bufs_es=4,
    enable_asserts=False,
):
    HD = 128
    SC = 512
    NK = D // HD        # proj contraction tiles
    NH = E // HD        # heads per core
    NSC = S // SC       # s-chunks / i-blocks
    NJ = S // HD        # attention j tiles
    NJG = NJ // 2       # j-tile pairs (exp batches)
    ND = D // SC        # WO n-chunks
    scale = 1.0 / math.sqrt(HD)

    nc = bacc.Bacc(
        "TRN2",
        target_bir_lowering=False,
        debug=False,
        enable_asserts=enable_asserts,
        num_devices=8,
    )

    # x is s-chunk-major: [HD, NSC * NK * SC]
    xr = nc.dram_tensor("xr", [HD, NSC * NK * SC], BF16,
                        kind="ExternalInput").ap()
    wqt = nc.dram_tensor("wqt", [HD, NK * E], BF16, kind="ExternalInput").ap()
    wkt = nc.dram_tensor("wkt", [HD, NK * E], BF16, kind="ExternalInput").ap()
    wvt = nc.dram_tensor("wvt", [HD, NK * E], BF16, kind="ExternalInput").ap()
    wot = nc.dram_tensor("wot", [HD, NH * D], BF16, kind="ExternalInput").ap()
    bqc = nc.dram_tensor("bqc", [HD, NH], F32, kind="ExternalInput").ap()
    bkc = nc.dram_tensor("bkc", [HD, NH], F32, kind="ExternalInput").ap()
    bvr = nc.dram_tensor("bvr", [1, E], BF16, kind="ExternalInput").ap()
    ones2d = nc.dram_tensor("ones2d", [HD, HD], BF16,
                            kind="ExternalInput").ap()
    out = nc.dram_tensor("out", [S, D], BF16, kind="ExternalOutput").ap()

    with tile.TileContext(nc) as tc:
        with (
            tc.tile_pool(name="qkv", bufs=1) as qkv_pool,
            tc.tile_pool(name="consts", bufs=1) as consts,
            tc.tile_pool(name="wbuf", bufs=1) as w_pool,
            tc.tile_pool(name="xroll", bufs=2) as x_pool,
        ):
            q_sb = qkv_pool.tile([HD, NH, S], BF16)
            k_sb = qkv_pool.tile([HD, NH, S], BF16)
            v_sb = qkv_pool.tile([HD, NJ, E], BF16)

            wq_sb = w_pool.tile([HD, NK, E], BF16, tag="wq")
            wk_sb = w_pool.tile([HD, NK, E], BF16, tag="wk")
            wv_sb = w_pool.tile([HD, NK, E], BF16, tag="wv")
            wo_sb = w_pool.tile([HD, NH, D], BF16, tag="wo")

            bq_sb = consts.tile([HD, NH], F32)
            bk_sb = consts.tile([HD, NH], F32)
            bv_sb = consts.tile([1, E], BF16)
            bv_bc = consts.tile([HD, E], BF16)
            allones = consts.tile([HD, HD], BF16)    # bcast-sum stationary

            # ---- DMA issue order = need order.  One HWDGE trigger queue
            # drains FIFO, so interleave per k-group: the kk=0..3 matmuls
            # of s-chunk 0 only need the first ~1.6MB.
            NG = NK // 4

            def load_w_g(dst, src, width, g):
                nc.sync.dma_start(
                    out=dst[:, 4 * g:4 * g + 4, :],
                    in_=src[:, 4 * g * width:(4 * g + 4) * width]
                    .rearrange("p (k e) -> p k e", e=width))

            x_tiles = [None] * NSC

            def load_x_g(si, g):
                if x_tiles[si] is None:
                    x_tiles[si] = x_pool.tile([HD, NK, SC], BF16, tag="x",
                                              name=f"x{si}")
                off = si * NK * SC + 4 * g * SC
                nc.sync.dma_start(
                    out=x_tiles[si][:, 4 * g:4 * g + 4, :],
                    in_=xr[:, off:off + 4 * SC].rearrange(
                        "p (k s) -> p k s", s=SC))

            for g in range(NG):
                load_w_g(wq_sb, wqt, E, g)
                load_w_g(wk_sb, wkt, E, g)
                load_x_g(0, g)
            nc.sync.dma_start(out=bq_sb, in_=bqc)
            nc.sync.dma_start(out=bk_sb, in_=bkc)
            nc.sync.dma_start(out=bv_sb, in_=bvr)
            nc.sync.dma_start(out=allones, in_=ones2d)
            for g in range(NG):
                load_w_g(wv_sb, wvt, E, g)
            for g in range(NG):
                load_x_g(1, g)
            nc.sync.dma_start(
                out=wo_sb, in_=wot.rearrange("p (k d) -> p k d", d=D))
            nc.gpsimd.partition_broadcast(bv_bc, bv_sb)

            # -------- Phase A: fused Q,K,V projections per s-chunk --------
            with tc.tile_pool(name="psAB", bufs=1, space="PSUM") as psAB:
                for si in range(NSC):
                    s0 = si * SC
                    xt = x_tiles[si]
                    xv = xt.rearrange("p k (t h) -> p k t h", h=HD)
                    psQ = [psAB.tile([HD, SC], F32, tag=f"q{m}",
                                     name=f"psq{m}") for m in range(NH)]
                    psK = [psAB.tile([HD, SC], F32, tag=f"k{m}",
                                     name=f"psk{m}") for m in range(NH)]
                    for kk in range(NK):
                        st = kk == 0
                        sp = kk == NK - 1
                        for m in range(NH):
                            nc.tensor.matmul(
                                psQ[m],
                                wq_sb[:, kk, m * HD:(m + 1) * HD],
                                xt[:, kk, :],
                                start=st, stop=sp,
                            )
                            nc.tensor.matmul(
                                psK[m],
                                wk_sb[:, kk, m * HD:(m + 1) * HD],
                                xt[:, kk, :],
                                start=st, stop=sp,
                            )
                    for m in range(NH):
                        nc.vector.tensor_scalar_add(
                            q_sb[:, m, s0:s0 + SC], psQ[m],
                            bq_sb[:, m:m + 1])
                        nc.vector.tensor_scalar_add(
                            k_sb[:, m, s0:s0 + SC], psK[m],
                            bk_sb[:, m:m + 1])
                    # prefetch x for si+2 now that chunk si is nearly done
                    if si + 2 < NSC:
                        for g in range(NG):
                            load_x_g(si + 2, g)
                    # V for this chunk: x stationary, wv moving
                    psV = [psAB.tile([HD, E], F32, tag=f"q{mv}",
                                     name=f"psv{mv}") for mv in range(NH)]
                    for kk in range(NK):
                        for mv in range(NH):
                            nc.tensor.matmul(
                                psV[mv],
                                xv[:, kk, mv, :],
                                wv_sb[:, kk, :],
                                start=(kk == 0), stop=(kk == NK - 1),
                            )
                    for mv in range(NH):
                        nc.vector.tensor_add(
                            v_sb[:, si * NH + mv, :], psV[mv], bv_bc)

            # ------- Phase C/D: attention with WO interleaved per ib -------
            with (
                tc.tile_pool(name="outT", bufs=1) as outT_pool,
                tc.tile_pool(name="attws", bufs=2) as ws_pool,
                tc.tile_pool(name="es", bufs=bufs_es) as es_pool,
                tc.tile_pool(name="esp", bufs=4) as esp_pool,
                tc.tile_pool(name="og", bufs=2) as og_pool,
                tc.tile_pool(name="psS", bufs=2, space="PSUM") as psS_pool,
                tc.tile_pool(name="psOW", bufs=3, space="PSUM") as psOW_pool,
                tc.tile_pool(name="psN", bufs=1, space="PSUM") as psN_pool,
            ):
                outT_sb = outT_pool.tile([HD, NH, S], BF16)
                for ib in range(NSC):
                    i0 = ib * SC
                    for h in range(NH):
                        psO = psOW_pool.tile([HD, SC], F32, tag="ow",
                                             name="psO")
                        ps_bc = psN_pool.tile([HD, SC], F32, tag="bc")
                        es_t = [None] * NJG
                        esp_t = [None] * NJG

                        def emit_av(jg, h=h, psO=psO, es_t=es_t):
                            for u in range(2):
                                j = 2 * jg + u
                                nc.tensor.matmul(
                                    psO,
                                    v_sb[:, j, h * HD:(h + 1) * HD],
                                    es_t[jg][:, u, :],
                                    start=(j == 0), stop=(j == NJ - 1),
                                )

                        def emit_bc(jg, ps_bc=ps_bc, esp_t=esp_t):
                            nc.tensor.matmul(
                                ps_bc, allones, esp_t[jg],
                                start=(jg == 0), stop=(jg == NJG - 1),
                            )

                        for jg in range(NJG):
                            psS = psS_pool.tile([HD, 2, SC], F32, tag="s")
                            for u in range(2):
                                j = 2 * jg + u
                                nc.tensor.matmul(
                                    psS[:, u, :],
                                    k_sb[:, h, j * HD:(j + 1) * HD],
                                    q_sb[:, h, i0:i0 + SC],
                                    start=True, stop=True,
                                )
                            if jg >= 1:
                                emit_av(jg - 1)
                            if jg >= 2:
                                emit_bc(jg - 2)
                            es = es_pool.tile([HD, 2, SC], BF16, tag="es",
                                              name="es")
                            nc.scalar.activation(es, psS, AF.Exp,
                                                 scale=scale)
                            es_t[jg] = es
                            # pair-sum of the two exp tiles on DVE (bf16,
                            # 2x rate); the all-ones matmuls then reduce
                            # the 8 pair tiles into the softmax denominator
                            esp = esp_pool.tile([HD, SC], BF16, tag="esp",
                                                name="esp")
                            nc.vector.tensor_add(esp, es[:, 0, :],
                                                 es[:, 1, :])
                            esp_t[jg] = esp
                        emit_av(NJG - 1)
                        emit_bc(NJG - 2)
                        emit_bc(NJG - 1)
                        recip = ws_pool.tile([HD, SC], F32, tag="recip")
                        nc.vector.reciprocal_approx_fast(recip, ps_bc)
                        nc.vector.tensor_mul(
                            outT_sb[:, h, i0:i0 + SC], psO, recip)

                    # WO projection for this ib's four i-tiles
                    for t in range(NSC):
                        it = ib * NSC + t
                        og = og_pool.tile([HD, D], BF16, tag="og")
                        for nn in range(ND):
                            psW = psOW_pool.tile([HD, SC], F32, tag="ow",
                                                 name="psW")
                            for kk in range(NH):
                                nc.tensor.matmul(
                                    psW,
                                    outT_sb[:, kk, it * HD:(it + 1) * HD],
                                    wo_sb[:, kk, nn * SC:(nn + 1) * SC],
                                    start=(kk == 0), stop=(kk == NH - 1),
                                )
                            if nn % 2 == 0:
                                nc.scalar.copy(
                                    og[:, nn * SC:(nn + 1) * SC], psW)
                            else:
                                nc.vector.tensor_copy(
                                    og[:, nn * SC:(nn + 1) * SC], psW)
                        nc.sync.dma_start(
                            out=out[it * HD:(it + 1) * HD, :], in_=og)

    nc.compile()
    return nc


# ---------------------------------------------------------------------------
# Host-side sharding helpers
# ---------------------------------------------------------------------------

def _bf16(a):
    import ml_dtypes
    return np.asarray(a).astype(ml_dtypes.bfloat16)


def make_in_map(x_b, wq_e, bq_e, wk_e, bk_e, wv_e, bv_e, wo_e):
    """Per-core input dict. x_b [S, D]; w*_e [E, D] row slices; wo_e [D, E]
    column slice; b*_e [E]."""
    E = wq_e.shape[0]
    S, D = x_b.shape
    HD = 128
    SC = 512
    NH = E // HD
    NK = D // HD
    NSC = S // SC

    def wrelayout(wT):  # [D, E'] -> [HD, NK*E'] with k-tile-major columns
        Ew = wT.shape[1]
        return _bf16(
            wT.reshape(NK, HD, Ew).transpose(1, 0, 2).reshape(HD, NK * Ew))

    xT = x_b.T  # [D, S]
    # s-chunk-major x: xr[hd, si, k, s] = xT[k*HD+hd, si*SC+s]
    xr = xT.reshape(NK, HD, NSC, SC).transpose(1, 2, 0, 3).reshape(HD, -1)
    return {
        "xr": _bf16(xr),
        "wqt": wrelayout(wq_e.T),
        "wkt": wrelayout(wk_e.T),
        "wvt": wrelayout(wv_e.T),
        "wot": _bf16(
            wo_e.T.reshape(NH, HD, D).transpose(1, 0, 2).reshape(HD, NH * D)),
        "bqc": np.ascontiguousarray(bq_e.reshape(NH, HD).T),
        "bkc": np.ascontiguousarray(bk_e.reshape(NH, HD).T),
        "bvr": _bf16(bv_e.reshape(1, E)),
        "ones2d": _bf16(np.ones((HD, HD), np.float32)),
    }


def core_reference(x_b, wq_e, bq_e, wk_e, bk_e, wv_e, bv_e, wo_e):
    """Numpy reference for one core's partial output."""
    HD = 128
    q = x_b @ wq_e.T + bq_e
    k = x_b @ wk_e.T + bk_e
    v = x_b @ wv_e.T + bv_e
    E = q.shape[1]
    outs = []
    for h in range(E // HD):
        qh = q[:, h * HD:(h + 1) * HD]
        kh = k[:, h * HD:(h + 1) * HD]
        vh = v[:, h * HD:(h + 1) * HD]
        s = (qh @ kh.T) / math.sqrt(HD)
        p = np.exp(s)
        outs.append((p @ vh) / p.sum(-1, keepdims=True))
    o = np.concatenate(outs, axis=1)
    return o @ wo_e.T


# ---------------------------------------------------------------------------
# Entry point: full-input kernel with internal 8-way sharding
# ---------------------------------------------------------------------------

import os as _os

_NC_CACHE = {}


def _get_module():
    if "nc" not in _NC_CACHE:
        _NC_CACHE["nc"] = build_module(S=2048, D=2048, E=512)
    return _NC_CACHE["nc"]


def kernel(x, wq, bq, wk, bk, wv, bv, wo, bo):
    """Full inputs -> full output. 8 cores = 2 (batch) x 4 (head-group)."""
    from concourse import bass_utils

    x = np.asarray(x, dtype=np.float32)
    wq, bq = np.asarray(wq, np.float32), np.asarray(bq, np.float32)
    wk, bk = np.asarray(wk, np.float32), np.asarray(bk, np.float32)
    wv, bv = np.asarray(wv, np.float32), np.asarray(bv, np.float32)
    wo, bo = np.asarray(wo, np.float32), np.asarray(bo, np.float32)

    E = 512
    nc = _get_module()
    in_maps = []
    for c in range(8):
        b, g = divmod(c, 4)
        e = slice(g * E, (g + 1) * E)
        in_maps.append(make_in_map(
            x[b], wq[e], bq[e], wk[e], bk[e], wv[e], bv[e], wo[:, e]))

    trace = bool(int(_os.environ.get("ATTN_TRACE", "0")))
    kw = {}
    if trace:
        tmpdir = _os.environ.get("ATTN_TRACE_DIR") or None
        kw = dict(trace=True, tmpdir=tmpdir, trace_cores=[0])
    res = bass_utils.run_bass_kernel_spmd(
        nc, in_maps, core_ids=list(range(8)), **kw)
    if trace:
        print(f"HW exec time: {res.exec_time_ns} ns")
        _NC_CACHE["last_results"] = res

    y = np.empty((2, 2048, 2048), np.float32)
    for b in range(2):
        acc = np.asarray(res.results[4 * b]["out"], np.float32)
        for g in range(1, 4):
            acc += np.asarray(res.results[4 * b + g]["out"], np.float32)
        y[b] = acc + bo
    return y
